# revision 5
# baseline (speedup 1.0000x reference)
"""GIN encoder (3-layer) on 8 Trainium2 NeuronCores.

Sharding: nodes partitioned into 8 contiguous ranges (by dst); each core
processes the edges targeting its nodes.  Per layer:
  - per-edge messages h[src] gathered from a full replica of h in local HBM
    via gpsimd dma_gather (int16 indices -> two overlapping 32768-row windows)
  - segment-sum by dst via one-hot matmuls accumulated in PSUM
    (out = msgs.T @ E_mat gives transposed aggregates)
  - MLP with stationary weights: W.T @ zT, per-partition bias+ReLU on ACT
  - per-graph pooling via one-hot matmul (batch is sorted)
  - new h blocks pushed to outputs; AllGather rebuilds the full replica
    between layers.
"""
import os
import sys

sys.path.insert(0, '/opt/trn_rl_repo')

import numpy as np
from contextlib import ExitStack

from concourse import bacc, mybir, tile
from concourse.bass_utils import run_bass_kernel_spmd
from concourse.masks import make_identity

P = 128
D = 128
NCORES = 8
L = 3
WIN = 32768  # int16-addressable window rows

LAST_RESULT = None  # test harness can read exec_time_ns from here


# ---------------------------------------------------------------- host prep
def _prep(src, dst, batch, n_nodes, npad):
    """Partition edges by dst range, build padded per-block chunk streams."""
    ncore_nodes = npad // NCORES
    bpc = ncore_nodes // P
    nblk = NCORES * bpc
    single_win = npad <= WIN
    hi_start = 0 if single_win else npad - WIN

    blk = dst // P                      # global block of each edge
    is_hi = np.zeros_like(src, dtype=bool) if single_win else (src >= WIN)
    key = blk * 2 + is_hi
    order = np.argsort(key, kind='stable')
    src_s, dst_s, key_s = src[order], dst[order], key[order]
    cnt = np.bincount(key_s, minlength=nblk * 2)
    offs = np.concatenate([[0], np.cumsum(cnt)])

    k_lo = max(1, int(np.ceil(cnt[0::2].max() / P)))
    k_hi = 0 if single_win else int(np.ceil(cnt[1::2].max() / P))

    cores = []
    for c in range(NCORES):
        idx_segs, dst_cols = [], []
        for b in range(bpc):
            g = c * bpc + b
            blk_start = g * P
            for half, k in ((0, k_lo), (1, k_hi)):
                if k == 0:
                    continue
                e0, e1 = offs[2 * g + half], offs[2 * g + half + 1]
                n = e1 - e0
                cap = k * P
                idx = np.zeros(cap, np.int16)
                dl = np.full(cap, -1.0, np.float32)
                s = src_s[e0:e1]
                idx[:n] = (s if half == 0 else s - hi_start).astype(np.int16)
                dl[:n] = (dst_s[e0:e1] - blk_start).astype(np.float32)
                # wrap this segment: position i -> [i%16, i//16]
                idx_segs.append(idx.reshape(-1, 16).T)         # [16, cap/16]
                dst_cols.append(dl.reshape(-1, P).T)            # [P, k]
        idx_stream = np.tile(np.concatenate(idx_segs, axis=1), (8, 1))
        dst_stream = np.concatenate(dst_cols, axis=1)
        # batch_local per block column; pad nodes (>= n_nodes) -> -1
        nodes = c * ncore_nodes + np.arange(ncore_nodes)
        g_lo = int(batch[min(c * ncore_nodes, n_nodes - 1)])
        bl = np.where(nodes < n_nodes, batch[np.minimum(nodes, n_nodes - 1)] - g_lo,
                      -1.0).astype(np.float32)
        n_graphs = int(bl[bl >= 0].max()) + 1 if (bl >= 0).any() else 0
        assert n_graphs <= P, n_graphs
        batch_loc = bl.reshape(bpc, P).T.copy()                 # [P, bpc]
        cores.append(dict(idx=np.ascontiguousarray(idx_stream),
                          dstl=np.ascontiguousarray(dst_stream),
                          bloc=np.ascontiguousarray(batch_loc),
                          g_lo=g_lo, n_graphs=n_graphs))
    return cores, k_lo, k_hi, bpc, single_win, hi_start


# ---------------------------------------------------------------- program
def _build(npad, bpc, k_lo, k_hi, single_win, hi_start):
    kt = k_lo + k_hi
    npc = bpc * P
    f32 = mybir.dt.float32
    nc = bacc.Bacc("TRN2", target_bir_lowering=False, debug=False,
                   num_devices=NCORES)

    x_full = nc.declare_dram_parameter("x_full", [npad, D], f32, isOutput=False)
    xT_own = nc.declare_dram_parameter("xT_own", [P, npc], f32, isOutput=False)
    idx_in = nc.declare_dram_parameter("idx", [P, bpc * kt * 8], mybir.dt.int16,
                                       isOutput=False)
    dst_in = nc.declare_dram_parameter("dstl", [P, bpc * kt], f32, isOutput=False)
    bloc_in = nc.declare_dram_parameter("bloc", [P, bpc], f32, isOutput=False)
    w1_in = nc.declare_dram_parameter("W1", [L, D, D], f32, isOutput=False)
    w2_in = nc.declare_dram_parameter("W2", [L, D, D], f32, isOutput=False)
    b1_in = nc.declare_dram_parameter("b1T", [P, L], f32, isOutput=False)
    b2_in = nc.declare_dram_parameter("b2T", [P, L], f32, isOutput=False)
    iota_in = nc.declare_dram_parameter("iota", [P, P], f32, isOutput=False)
    y_out = [nc.declare_dram_parameter(f"y{l}", [npc, D], f32, isOutput=True)
             for l in range(L)]
    pool_out = [nc.declare_dram_parameter(f"pool{l}", [P, D], f32, isOutput=True)
                for l in range(L)]

    relu = mybir.ActivationFunctionType.Relu
    L_RUN = 1 if os.environ.get("KV_ONE_LAYER") else L
    SKIP_COLL = bool(os.environ.get("KV_SKIP_COLL"))
    SKIP_TR = bool(os.environ.get("KV_SKIP_TR"))
    POOL_NOACC = bool(os.environ.get("KV_POOL_NOACC"))

    with tile.TileContext(nc) as tc, ExitStack() as ctx:
        cpool = ctx.enter_context(tc.tile_pool(name="const", bufs=1))
        hpool = ctx.enter_context(tc.tile_pool(name="h", bufs=1))
        mpool = ctx.enter_context(tc.tile_pool(name="msgs", bufs=3))
        epool = ctx.enter_context(tc.tile_pool(name="emat", bufs=3))
        zpool = ctx.enter_context(tc.tile_pool(name="z", bufs=3))
        agg_pp = ctx.enter_context(tc.tile_pool(name="aggp", bufs=2, space="PSUM"))
        mlp_pp = ctx.enter_context(tc.tile_pool(name="mlpp", bufs=1, space="PSUM"))
        tr_pp = ctx.enter_context(tc.tile_pool(name="trp", bufs=1, space="PSUM"))
        pool_pp = ctx.enter_context(tc.tile_pool(name="poolp", bufs=1, space="PSUM"))
        dpool = ctx.enter_context(tc.tile_pool(name="dram", bufs=1, space="DRAM"))

        idx_sb = cpool.tile([P, bpc * kt * 8], mybir.dt.int16)
        nc.sync.dma_start(idx_sb[:], idx_in[:])
        dst_sb = cpool.tile([P, bpc * kt], f32)
        nc.sync.dma_start(dst_sb[:], dst_in[:])
        bloc_sb = cpool.tile([P, bpc], f32)
        nc.sync.dma_start(bloc_sb[:], bloc_in[:])
        iota_sb = cpool.tile([P, P], f32)
        nc.sync.dma_start(iota_sb[:], iota_in[:])
        ident_sb = cpool.tile([P, P], f32)
        make_identity(nc, ident_sb[:])
        w1_sb = []
        w2_sb = []
        for l in range(L):
            wt1 = cpool.tile([P, D], f32, tag=f"w1_{l}", name=f"w1_{l}")
            wt2 = cpool.tile([P, D], f32, tag=f"w2_{l}", name=f"w2_{l}")
            w1_sb.append(wt1)
            w2_sb.append(wt2)
        for l in range(L):
            nc.sync.dma_start(w1_sb[l][:], w1_in[l])
            nc.sync.dma_start(w2_sb[l][:], w2_in[l])
        b1_sb = cpool.tile([P, L], f32)
        nc.sync.dma_start(b1_sb[:], b1_in[:])
        b2_sb = cpool.tile([P, L], f32)
        nc.sync.dma_start(b2_sb[:], b2_in[:])

        hT_a = hpool.tile([P, npc], f32, tag="hta")
        hT_b = hpool.tile([P, npc], f32, tag="htb")
        hT = [hT_a, hT_b]
        nc.sync.dma_start(hT[0][:], xT_own[:])

        hown = []
        hfull = []
        for l in range(L - 1):
            ho = dpool.tile([npc, D], f32, tag=f"hown{l}", name=f"hown{l}")
            hf_t = dpool.tile([npad, D], f32, tag=f"hfull{l}", name=f"hfull{l}")
            hown.append(ho)
            hfull.append(hf_t)

        for l in range(L_RUN):
            hT_cur, hT_nxt = hT[l % 2], hT[(l + 1) % 2]
            pooled_ps = pool_pp.tile([P, D], f32, space="PSUM", tag="pool")
            for b in range(bpc):
                # ---- gather messages for this block's edges
                if l == 0:
                    src_lo = x_full[0:min(WIN, npad), :]
                    src_hi = None if single_win else x_full[hi_start:npad, :]
                else:
                    hf = hfull[l - 1]
                    src_lo = hf[0:min(WIN, npad), :]
                    src_hi = None if single_win else hf[hi_start:npad, :]
                seg8 = b * kt * 8
                msgs_lo = mpool.tile([P, k_lo, D], f32, tag="mlo")
                nc.gpsimd.dma_gather(
                    out_ap=msgs_lo[:], in_ap=src_lo,
                    idxs_ap=idx_sb[:, seg8:seg8 + k_lo * 8],
                    num_idxs=k_lo * P, num_idxs_reg=k_lo * P, elem_size=D,
                    single_packet=False)
                if k_hi:
                    msgs_hi = mpool.tile([P, k_hi, D], f32, tag="mhi")
                    nc.gpsimd.dma_gather(
                        out_ap=msgs_hi[:], in_ap=src_hi,
                        idxs_ap=idx_sb[:, seg8 + k_lo * 8:seg8 + kt * 8],
                        num_idxs=k_hi * P, num_idxs_reg=k_hi * P, elem_size=D,
                        single_packet=False)

                # ---- segment-sum: aggT[d, n] += sum_j msgs[j, d] * Emat[j, n]
                agg_ps = agg_pp.tile([P, P], f32, space="PSUM", tag="agg")
                for k in range(kt):
                    col = b * kt + k
                    emat = epool.tile([P, P], f32, tag="emat")
                    nc.vector.tensor_tensor(
                        out=emat[:],
                        in0=dst_sb[:, col:col + 1].to_broadcast([P, P])[:],
                        in1=iota_sb[:],
                        op=mybir.AluOpType.is_equal)
                    rhs = (msgs_lo[:, k, :] if k < k_lo
                           else msgs_hi[:, k - k_lo, :])
                    nc.tensor.matmul(out=agg_ps[:], lhsT=rhs, rhs=emat[:],
                                     start=(k == 0), stop=(k == kt - 1))

                # ---- zT = hT + aggT ; MLP (transposed); relu
                zT = zpool.tile([P, P], f32, tag="zT")
                nc.vector.tensor_add(zT[:], agg_ps[:],
                                     hT_cur[:, b * P:(b + 1) * P])
                mm1 = mlp_pp.tile([P, P], f32, space="PSUM", tag="mm1")
                nc.tensor.matmul(out=mm1[:], lhsT=w1_sb[l][:], rhs=zT[:],
                                 start=True, stop=True)
                a1T = zpool.tile([P, P], f32, tag="a1T")
                nc.scalar.activation(a1T[:], mm1[:], relu, bias=b1_sb[:, l:l + 1])
                mm2 = mlp_pp.tile([P, P], f32, space="PSUM", tag="mm2")
                nc.tensor.matmul(out=mm2[:], lhsT=w2_sb[l][:], rhs=a1T[:],
                                 start=True, stop=True)
                nc.scalar.activation(hT_nxt[:, b * P:(b + 1) * P], mm2[:], relu,
                                     bias=b2_sb[:, l:l + 1])

                # ---- natural-layout h block (for DRAM rows + pooling)
                hnew = zpool.tile([P, P], f32, tag="hnew")
                if SKIP_TR:
                    nc.vector.tensor_copy(hnew[:], hT_nxt[:, b * P:(b + 1) * P])
                else:
                    tr_ps = tr_pp.tile([P, P], f32, space="PSUM", tag="tr")
                    nc.tensor.transpose(out=tr_ps[:],
                                        in_=hT_nxt[:, b * P:(b + 1) * P],
                                        identity=ident_sb[:])
                    nc.vector.tensor_copy(hnew[:], tr_ps[:])
                nc.sync.dma_start(y_out[l][b * P:(b + 1) * P, :], hnew[:])
                if l < L - 1:
                    nc.sync.dma_start(hown[l][b * P:(b + 1) * P, :], hnew[:])

                # ---- pooling: pooled[g, d] += sum_n Bmat[n, g] * hnew[n, d]
                bmat = epool.tile([P, P], f32, tag="bmat")
                nc.vector.tensor_tensor(
                    out=bmat[:],
                    in0=bloc_sb[:, b:b + 1].to_broadcast([P, P])[:],
                    in1=iota_sb[:],
                    op=mybir.AluOpType.is_equal)
                nc.tensor.matmul(out=pooled_ps[:], lhsT=bmat[:], rhs=hnew[:],
                                 start=(b == 0) or POOL_NOACC,
                                 stop=(b == bpc - 1) or POOL_NOACC)

            pooled_sb = zpool.tile([P, D], f32, tag="pooled")
            nc.vector.tensor_copy(pooled_sb[:], pooled_ps[:])
            nc.sync.dma_start(pool_out[l][:], pooled_sb[:])

            if l < L_RUN - 1:
                if SKIP_COLL:
                    nc.sync.dma_start(hfull[l][0:npc, :], hown[l][:])
                else:
                    nc.gpsimd.collective_compute(
                        "AllGather", mybir.AluOpType.bypass,
                        replica_groups=[list(range(NCORES))],
                        ins=[hown[l].opt()], outs=[hfull[l].opt()])

    nc.finalize()
    return nc


# ---------------------------------------------------------------- entry
def kernel(x, edge_index, batch, W1, b1, W2, b2):
    global LAST_RESULT
    x = np.asarray(x, np.float32)
    edge_index = np.asarray(edge_index, np.int64)
    batch = np.asarray(batch, np.int64)
    W1 = np.ascontiguousarray(np.asarray(W1, np.float32))
    W2 = np.ascontiguousarray(np.asarray(W2, np.float32))
    b1 = np.asarray(b1, np.float32)
    b2 = np.asarray(b2, np.float32)

    n, d = x.shape
    assert d == D
    num_graphs = int(batch.max()) + 1
    npad = ((n + NCORES * P - 1) // (NCORES * P)) * (NCORES * P)
    npc = npad // NCORES

    src, dst = edge_index[0], edge_index[1]
    cores, k_lo, k_hi, bpc, single_win, hi_start = _prep(
        src, dst, batch, n, npad)

    nc = _build(npad, bpc, k_lo, k_hi, single_win, hi_start)

    x_pad = np.zeros((npad, D), np.float32)
    x_pad[:n] = x
    iota = np.tile(np.arange(P, dtype=np.float32), (P, 1))
    b1T = np.tile(b1.T[:, None, :], (1, 1, 1)).reshape(D, L)  # [D, L]
    b1T = np.ascontiguousarray(b1.T)                          # [D, L]
    b2T = np.ascontiguousarray(b2.T)

    in_maps = []
    for c in range(NCORES):
        cc = cores[c]
        xT = np.ascontiguousarray(x_pad[c * npc:(c + 1) * npc].T)
        in_maps.append(dict(x_full=x_pad, xT_own=xT, idx=cc['idx'],
                            dstl=cc['dstl'], bloc=cc['bloc'],
                            W1=W1, W2=W2, b1T=b1T, b2T=b2T, iota=iota))

    res = run_bass_kernel_spmd(nc, in_maps, list(range(NCORES)),
                               trace=bool(os.environ.get("BASS_TRACE")))
    LAST_RESULT = res

    xs = np.empty((L, n, D), np.float32)
    pooled = np.zeros((L, num_graphs, D), np.float32)
    for c in range(NCORES):
        r = res.results[c]
        lo, hi = c * npc, min((c + 1) * npc, n)
        for l in range(L):
            xs[l, lo:hi] = r[f"y{l}"][:hi - lo]
            g = cores[c]
            pooled[l, g['g_lo']:g['g_lo'] + g['n_graphs']] += \
                r[f"pool{l}"][:g['n_graphs']]
    xs_cat = np.concatenate(list(xs), axis=1)
    pooled_cat = np.concatenate(list(pooled), axis=1)
    return pooled_cat, xs_cat


# revision 8
# speedup vs baseline: 1.1040x; 1.1040x over previous
"""GIN encoder (3-layer) on 8 Trainium2 NeuronCores.

Sharding: nodes partitioned into 8 contiguous ranges (by dst); each core
processes the edges targeting its nodes.  Per layer:
  - per-edge messages h[src] gathered from a full replica of h in local HBM
    via gpsimd dma_gather (int16 indices -> two overlapping 32768-row windows)
  - segment-sum by dst via one-hot matmuls accumulated in PSUM
    (out = msgs.T @ E_mat gives transposed aggregates)
  - MLP with stationary weights: W.T @ zT, per-partition bias+ReLU on ACT
  - per-graph pooling via one-hot matmul (batch is sorted)
  - new h blocks pushed to outputs; AllGather rebuilds the full replica
    between layers.
"""
import os
import sys

sys.path.insert(0, '/opt/trn_rl_repo')

import numpy as np
from contextlib import ExitStack

from concourse import bacc, mybir, tile
from concourse.bass_utils import run_bass_kernel_spmd
from concourse.masks import make_identity

P = 128
D = 128
NCORES = 8
L = 3
WIN = 32768  # int16-addressable window rows

LAST_RESULT = None  # test harness can read exec_time_ns from here


# ---------------------------------------------------------------- host prep
def _prep(src, dst, batch, n_nodes, npad):
    """Partition edges by dst range, build padded per-block chunk streams."""
    ncore_nodes = npad // NCORES
    bpc = ncore_nodes // P
    nblk = NCORES * bpc
    single_win = npad <= WIN
    hi_start = 0 if single_win else npad - WIN

    # permute node ids so each per-quarter AllGather output is contiguous:
    # new id = [quarter][core][row within quarter]
    qn = (bpc + 3) // 4
    qstarts = np.array([min(i * qn * P, ncore_nodes) for i in range(5)])
    qlens = np.diff(qstarts)
    qbase8 = np.concatenate([[0], np.cumsum(qlens * NCORES)])

    def perm_ids(r):
        c, local = r // ncore_nodes, r % ncore_nodes
        q = np.searchsorted(qstarts, local, side='right') - 1
        return qbase8[q] + c * qlens[q] + (local - qstarts[q])

    psrc = perm_ids(src)
    blk = dst // P                      # global block of each edge
    is_hi = np.zeros_like(src, dtype=bool) if single_win else (psrc >= WIN)
    key = blk * 2 + is_hi
    order = np.argsort(key, kind='stable')
    src_s, dst_s, key_s = psrc[order], dst[order], key[order]
    cnt = np.bincount(key_s, minlength=nblk * 2)
    offs = np.concatenate([[0], np.cumsum(cnt)])

    k_lo = max(1, int(np.ceil(cnt[0::2].max() / P)))
    k_hi = 0 if single_win else int(np.ceil(cnt[1::2].max() / P))

    cores = []
    for c in range(NCORES):
        idx_segs, dst_cols = [], []
        for b in range(bpc):
            g = c * bpc + b
            blk_start = g * P
            for half, k in ((0, k_lo), (1, k_hi)):
                if k == 0:
                    continue
                e0, e1 = offs[2 * g + half], offs[2 * g + half + 1]
                n = e1 - e0
                cap = k * P
                idx = np.zeros(cap, np.int16)
                dl = np.full(cap, -1.0, np.float32)
                s = src_s[e0:e1]
                idx[:n] = (s if half == 0 else s - hi_start).astype(np.int16)
                dl[:n] = (dst_s[e0:e1] - blk_start).astype(np.float32)
                # wrap this segment: position i -> [i%16, i//16]
                idx_segs.append(idx.reshape(-1, 16).T)         # [16, cap/16]
                dst_cols.append(dl.reshape(-1, P).T)            # [P, k]
        idx_stream = np.tile(np.concatenate(idx_segs, axis=1), (8, 1))
        dst_stream = np.concatenate(dst_cols, axis=1)
        # batch_local per block column; pad nodes (>= n_nodes) -> -1
        nodes = c * ncore_nodes + np.arange(ncore_nodes)
        g_lo = int(batch[min(c * ncore_nodes, n_nodes - 1)])
        bl = np.where(nodes < n_nodes, batch[np.minimum(nodes, n_nodes - 1)] - g_lo,
                      -1.0).astype(np.float32)
        n_graphs = int(bl[bl >= 0].max()) + 1 if (bl >= 0).any() else 0
        assert n_graphs <= P, n_graphs
        batch_loc = bl.reshape(bpc, P).T.copy()                 # [P, bpc]
        cores.append(dict(idx=np.ascontiguousarray(idx_stream),
                          dstl=np.ascontiguousarray(dst_stream),
                          bloc=np.ascontiguousarray(batch_loc),
                          g_lo=g_lo, n_graphs=n_graphs))
    all_perm = perm_ids(np.arange(npad))
    return cores, k_lo, k_hi, bpc, single_win, hi_start, all_perm, qstarts


# ---------------------------------------------------------------- program
def _build(npad, bpc, k_lo, k_hi, single_win, hi_start):
    kt = k_lo + k_hi
    npc = bpc * P
    f32 = mybir.dt.float32
    nc = bacc.Bacc("TRN2", target_bir_lowering=False, debug=False,
                   num_devices=NCORES)

    x_full = nc.declare_dram_parameter("x_full", [npad, D], f32, isOutput=False)
    xT_own = nc.declare_dram_parameter("xT_own", [P, npc], f32, isOutput=False)
    idx_in = nc.declare_dram_parameter("idx", [P, bpc * kt * 8], mybir.dt.int16,
                                       isOutput=False)
    dst_in = nc.declare_dram_parameter("dstl", [P, bpc * kt], f32, isOutput=False)
    bloc_in = nc.declare_dram_parameter("bloc", [P, bpc], f32, isOutput=False)
    w1_in = nc.declare_dram_parameter("W1", [L, D, D], f32, isOutput=False)
    w2_in = nc.declare_dram_parameter("W2", [L, D, D], f32, isOutput=False)
    b1_in = nc.declare_dram_parameter("b1T", [P, L], f32, isOutput=False)
    b2_in = nc.declare_dram_parameter("b2T", [P, L], f32, isOutput=False)
    iota_in = nc.declare_dram_parameter("iota", [P, P], f32, isOutput=False)
    y_out = [nc.declare_dram_parameter(f"y{l}", [npc, D], f32, isOutput=True)
             for l in range(L)]
    pool_out = [nc.declare_dram_parameter(f"pool{l}", [P, D], f32, isOutput=True)
                for l in range(L)]

    relu = mybir.ActivationFunctionType.Relu
    L_RUN = 1 if os.environ.get("KV_ONE_LAYER") else L
    SKIP_COLL = bool(os.environ.get("KV_SKIP_COLL"))
    SKIP_TR = bool(os.environ.get("KV_SKIP_TR"))
    POOL_NOACC = bool(os.environ.get("KV_POOL_NOACC"))

    with tile.TileContext(nc) as tc, ExitStack() as ctx:
        cpool = ctx.enter_context(tc.tile_pool(name="const", bufs=1))
        hpool = ctx.enter_context(tc.tile_pool(name="h", bufs=1))
        mpool = ctx.enter_context(tc.tile_pool(name="msgs", bufs=3))
        epool = ctx.enter_context(tc.tile_pool(name="emat", bufs=3))
        zpool = ctx.enter_context(tc.tile_pool(name="z", bufs=3))
        agg_pp = ctx.enter_context(tc.tile_pool(name="aggp", bufs=2, space="PSUM"))
        mlp_pp = ctx.enter_context(tc.tile_pool(name="mlpp", bufs=1, space="PSUM"))
        tr_pp = ctx.enter_context(tc.tile_pool(name="trp", bufs=1, space="PSUM"))
        pool_pp = ctx.enter_context(tc.tile_pool(name="poolp", bufs=1, space="PSUM"))
        dpool = ctx.enter_context(tc.tile_pool(name="dram", bufs=1, space="DRAM"))

        idx_sb = cpool.tile([P, bpc * kt * 8], mybir.dt.int16)
        nc.sync.dma_start(idx_sb[:], idx_in[:])
        dst_sb = cpool.tile([P, bpc * kt], f32)
        nc.sync.dma_start(dst_sb[:], dst_in[:])
        bloc_sb = cpool.tile([P, bpc], f32)
        nc.sync.dma_start(bloc_sb[:], bloc_in[:])
        iota_sb = cpool.tile([P, P], f32)
        nc.sync.dma_start(iota_sb[:], iota_in[:])
        ident_sb = cpool.tile([P, P], f32)
        make_identity(nc, ident_sb[:])
        w1_sb = []
        w2_sb = []
        for l in range(L):
            wt1 = cpool.tile([P, D], f32, tag=f"w1_{l}", name=f"w1_{l}")
            wt2 = cpool.tile([P, D], f32, tag=f"w2_{l}", name=f"w2_{l}")
            w1_sb.append(wt1)
            w2_sb.append(wt2)
        for l in range(L):
            nc.sync.dma_start(w1_sb[l][:], w1_in[l])
            nc.sync.dma_start(w2_sb[l][:], w2_in[l])
        b1_sb = cpool.tile([P, L], f32)
        nc.sync.dma_start(b1_sb[:], b1_in[:])
        b2_sb = cpool.tile([P, L], f32)
        nc.sync.dma_start(b2_sb[:], b2_in[:])

        hT_a = hpool.tile([P, npc], f32, tag="hta")
        hT_b = hpool.tile([P, npc], f32, tag="htb")
        hT = [hT_a, hT_b]
        nc.sync.dma_start(hT[0][:], xT_own[:])

        hown = []
        hfull = []
        for l in range(L - 1):
            ho = dpool.tile([npc, D], f32, tag=f"hown{l}", name=f"hown{l}")
            hf_t = dpool.tile([npad, D], f32, tag=f"hfull{l}", name=f"hfull{l}")
            hown.append(ho)
            hfull.append(hf_t)

        for l in range(L_RUN):
            hT_cur, hT_nxt = hT[l % 2], hT[(l + 1) % 2]
            pooled_ps = pool_pp.tile([P, D], f32, space="PSUM", tag="pool")
            for b in range(bpc):
                # ---- gather messages for this block's edges
                if l == 0:
                    src_lo = x_full[0:min(WIN, npad), :]
                    src_hi = None if single_win else x_full[hi_start:npad, :]
                else:
                    hf = hfull[l - 1]
                    src_lo = hf[0:min(WIN, npad), :]
                    src_hi = None if single_win else hf[hi_start:npad, :]
                seg8 = b * kt * 8
                msgs_lo = mpool.tile([P, k_lo, D], f32, tag="mlo")
                nc.gpsimd.dma_gather(
                    out_ap=msgs_lo[:], in_ap=src_lo,
                    idxs_ap=idx_sb[:, seg8:seg8 + k_lo * 8],
                    num_idxs=k_lo * P, num_idxs_reg=k_lo * P, elem_size=D,
                    single_packet=False)
                if k_hi:
                    msgs_hi = mpool.tile([P, k_hi, D], f32, tag="mhi")
                    nc.gpsimd.dma_gather(
                        out_ap=msgs_hi[:], in_ap=src_hi,
                        idxs_ap=idx_sb[:, seg8 + k_lo * 8:seg8 + kt * 8],
                        num_idxs=k_hi * P, num_idxs_reg=k_hi * P, elem_size=D,
                        single_packet=False)

                # ---- segment-sum: aggT[d, n] += sum_j msgs[j, d] * Emat[j, n]
                agg_ps = agg_pp.tile([P, P], f32, space="PSUM", tag="agg")
                for k in range(kt):
                    col = b * kt + k
                    emat = epool.tile([P, P], f32, tag="emat")
                    nc.vector.tensor_tensor(
                        out=emat[:],
                        in0=dst_sb[:, col:col + 1].to_broadcast([P, P])[:],
                        in1=iota_sb[:],
                        op=mybir.AluOpType.is_equal)
                    rhs = (msgs_lo[:, k, :] if k < k_lo
                           else msgs_hi[:, k - k_lo, :])
                    nc.tensor.matmul(out=agg_ps[:], lhsT=rhs, rhs=emat[:],
                                     start=(k == 0), stop=(k == kt - 1))

                # ---- zT = hT + aggT ; MLP (transposed); relu
                zT = zpool.tile([P, P], f32, tag="zT")
                nc.vector.tensor_add(zT[:], agg_ps[:],
                                     hT_cur[:, b * P:(b + 1) * P])
                mm1 = mlp_pp.tile([P, P], f32, space="PSUM", tag="mm1")
                nc.tensor.matmul(out=mm1[:], lhsT=w1_sb[l][:], rhs=zT[:],
                                 start=True, stop=True)
                a1T = zpool.tile([P, P], f32, tag="a1T")
                nc.scalar.activation(a1T[:], mm1[:], relu, bias=b1_sb[:, l:l + 1])
                mm2 = mlp_pp.tile([P, P], f32, space="PSUM", tag="mm2")
                nc.tensor.matmul(out=mm2[:], lhsT=w2_sb[l][:], rhs=a1T[:],
                                 start=True, stop=True)
                nc.scalar.activation(hT_nxt[:, b * P:(b + 1) * P], mm2[:], relu,
                                     bias=b2_sb[:, l:l + 1])

                # ---- natural-layout h block (for DRAM rows + pooling)
                hnew = zpool.tile([P, P], f32, tag="hnew")
                if SKIP_TR:
                    nc.vector.tensor_copy(hnew[:], hT_nxt[:, b * P:(b + 1) * P])
                else:
                    tr_ps = tr_pp.tile([P, P], f32, space="PSUM", tag="tr")
                    nc.tensor.transpose(out=tr_ps[:],
                                        in_=hT_nxt[:, b * P:(b + 1) * P],
                                        identity=ident_sb[:])
                    nc.vector.tensor_copy(hnew[:], tr_ps[:])
                nc.sync.dma_start(y_out[l][b * P:(b + 1) * P, :], hnew[:])
                if l < L - 1:
                    nc.sync.dma_start(hown[l][b * P:(b + 1) * P, :], hnew[:])

                # ---- pooling: pooled[g, d] += sum_n Bmat[n, g] * hnew[n, d]
                qsz = (bpc + 3) // 4
                if (l < L_RUN - 1 and not SKIP_COLL
                        and ((b + 1) % qsz == 0 or b == bpc - 1)):
                    r0 = (b // qsz) * qsz * P
                    r1 = min((b + 1) * P, npc)
                    nc.gpsimd.collective_compute(
                        "AllGather", mybir.AluOpType.bypass,
                        replica_groups=[list(range(NCORES))],
                        ins=[hown[l][r0:r1, :]],
                        outs=[hfull[l][NCORES * r0:NCORES * r1, :]])
                bmat = epool.tile([P, P], f32, tag="bmat")
                nc.vector.tensor_tensor(
                    out=bmat[:],
                    in0=bloc_sb[:, b:b + 1].to_broadcast([P, P])[:],
                    in1=iota_sb[:],
                    op=mybir.AluOpType.is_equal)
                nc.tensor.matmul(out=pooled_ps[:], lhsT=bmat[:], rhs=hnew[:],
                                 start=(b == 0) or POOL_NOACC,
                                 stop=(b == bpc - 1) or POOL_NOACC)

            pooled_sb = zpool.tile([P, D], f32, tag="pooled")
            nc.vector.tensor_copy(pooled_sb[:], pooled_ps[:])
            nc.sync.dma_start(pool_out[l][:], pooled_sb[:])

            if l < L_RUN - 1:
                if SKIP_COLL:
                    nc.sync.dma_start(hfull[l][0:npc, :], hown[l][:])
                else:
                    pass  # chunked allgather issued inside the block loop

    nc.finalize()
    return nc


# ---------------------------------------------------------------- entry
def kernel(x, edge_index, batch, W1, b1, W2, b2):
    global LAST_RESULT
    x = np.asarray(x, np.float32)
    edge_index = np.asarray(edge_index, np.int64)
    batch = np.asarray(batch, np.int64)
    W1 = np.ascontiguousarray(np.asarray(W1, np.float32))
    W2 = np.ascontiguousarray(np.asarray(W2, np.float32))
    b1 = np.asarray(b1, np.float32)
    b2 = np.asarray(b2, np.float32)

    n, d = x.shape
    assert d == D
    num_graphs = int(batch.max()) + 1
    npad = ((n + NCORES * P - 1) // (NCORES * P)) * (NCORES * P)
    npc = npad // NCORES

    src, dst = edge_index[0], edge_index[1]
    cores, k_lo, k_hi, bpc, single_win, hi_start, all_perm, qstarts = _prep(
        src, dst, batch, n, npad)

    nc = _build(npad, bpc, k_lo, k_hi, single_win, hi_start)

    x_pad = np.zeros((npad, D), np.float32)
    x_pad[:n] = x
    x_perm = np.zeros((npad, D), np.float32)
    x_perm[all_perm] = x_pad
    iota = np.tile(np.arange(P, dtype=np.float32), (P, 1))
    b1T = np.tile(b1.T[:, None, :], (1, 1, 1)).reshape(D, L)  # [D, L]
    b1T = np.ascontiguousarray(b1.T)                          # [D, L]
    b2T = np.ascontiguousarray(b2.T)

    in_maps = []
    for c in range(NCORES):
        cc = cores[c]
        xT = np.ascontiguousarray(x_pad[c * npc:(c + 1) * npc].T)
        in_maps.append(dict(x_full=x_perm, xT_own=xT, idx=cc['idx'],
                            dstl=cc['dstl'], bloc=cc['bloc'],
                            W1=W1, W2=W2, b1T=b1T, b2T=b2T, iota=iota))

    res = run_bass_kernel_spmd(nc, in_maps, list(range(NCORES)),
                               trace=bool(os.environ.get("BASS_TRACE")))
    LAST_RESULT = res

    xs = np.empty((L, n, D), np.float32)
    pooled = np.zeros((L, num_graphs, D), np.float32)
    for c in range(NCORES):
        r = res.results[c]
        lo, hi = c * npc, min((c + 1) * npc, n)
        for l in range(L):
            xs[l, lo:hi] = r[f"y{l}"][:hi - lo]
            g = cores[c]
            pooled[l, g['g_lo']:g['g_lo'] + g['n_graphs']] += \
                r[f"pool{l}"][:g['n_graphs']]
    xs_cat = np.concatenate(list(xs), axis=1)
    pooled_cat = np.concatenate(list(pooled), axis=1)
    return pooled_cat, xs_cat


# revision 9
# speedup vs baseline: 1.1990x; 1.0861x over previous
"""GIN encoder (3-layer) on 8 Trainium2 NeuronCores.

Sharding: nodes partitioned into 8 contiguous ranges (by dst); each core
processes the edges targeting its nodes.  Per layer:
  - per-edge messages h[src] gathered from a full replica of h in local HBM
    via gpsimd dma_gather (int16 indices -> two overlapping 32768-row windows)
  - segment-sum by dst via one-hot matmuls accumulated in PSUM
    (out = msgs.T @ E_mat gives transposed aggregates)
  - MLP with stationary weights: W.T @ zT, per-partition bias+ReLU on ACT
  - per-graph pooling via one-hot matmul (batch is sorted)
  - new h blocks pushed to outputs; AllGather rebuilds the full replica
    between layers.
"""
import os
import sys

sys.path.insert(0, '/opt/trn_rl_repo')

import numpy as np
from contextlib import ExitStack

from concourse import bacc, mybir, tile
from concourse.bass_utils import run_bass_kernel_spmd
from concourse.masks import make_identity

P = 128
D = 128
NCORES = 8
L = 3
WIN = 32768  # int16-addressable window rows

LAST_RESULT = None  # test harness can read exec_time_ns from here


# ---------------------------------------------------------------- host prep
def _prep(src, dst, batch, n_nodes, npad):
    """Partition edges by dst range, build padded per-block chunk streams."""
    ncore_nodes = npad // NCORES
    bpc = ncore_nodes // P
    nblk = NCORES * bpc
    single_win = npad <= WIN
    hi_start = 0 if single_win else npad - WIN

    # permute node ids so each per-quarter AllGather output is contiguous:
    # new id = [quarter][core][row within quarter]
    qn = (bpc + 3) // 4
    qstarts = np.array([min(i * qn * P, ncore_nodes) for i in range(5)])
    qlens = np.diff(qstarts)
    qbase8 = np.concatenate([[0], np.cumsum(qlens * NCORES)])

    def perm_ids(r):
        c, local = r // ncore_nodes, r % ncore_nodes
        q = np.searchsorted(qstarts, local, side='right') - 1
        return qbase8[q] + c * qlens[q] + (local - qstarts[q])

    psrc = perm_ids(src)
    blk = dst // P                      # global block of each edge
    is_hi = np.zeros_like(src, dtype=bool) if single_win else (psrc >= WIN)
    key = blk * 2 + is_hi
    order = np.argsort(key, kind='stable')
    src_s, dst_s, key_s = psrc[order], dst[order], key[order]
    cnt = np.bincount(key_s, minlength=nblk * 2)
    offs = np.concatenate([[0], np.cumsum(cnt)])

    cnt_lo = cnt[0::2].reshape(NCORES, bpc)
    cnt_hi = cnt[1::2].reshape(NCORES, bpc)
    k_lo = np.maximum(1, np.ceil(cnt_lo.max(axis=0) / P).astype(int))  # [bpc]
    k_hi = (np.zeros(bpc, int) if single_win
            else np.ceil(cnt_hi.max(axis=0) / P).astype(int))

    cores = []
    for c in range(NCORES):
        idx_segs, dst_cols = [], []
        for b in range(bpc):
            g = c * bpc + b
            blk_start = g * P
            for half, k in ((0, int(k_lo[b])), (1, int(k_hi[b]))):
                if k == 0:
                    continue
                e0, e1 = offs[2 * g + half], offs[2 * g + half + 1]
                n = e1 - e0
                cap = k * P
                idx = np.zeros(cap, np.int16)
                dl = np.full(cap, -1.0, np.float32)
                s = src_s[e0:e1]
                idx[:n] = (s if half == 0 else s - hi_start).astype(np.int16)
                dl[:n] = (dst_s[e0:e1] - blk_start).astype(np.float32)
                # wrap this segment: position i -> [i%16, i//16]
                idx_segs.append(idx.reshape(-1, 16).T)         # [16, cap/16]
                dst_cols.append(dl.reshape(-1, P).T)            # [P, k]
        idx_stream = np.tile(np.concatenate(idx_segs, axis=1), (8, 1))
        dst_stream = np.concatenate(dst_cols, axis=1)
        # batch_local per block column; pad nodes (>= n_nodes) -> -1
        nodes = c * ncore_nodes + np.arange(ncore_nodes)
        g_lo = int(batch[min(c * ncore_nodes, n_nodes - 1)])
        bl = np.where(nodes < n_nodes, batch[np.minimum(nodes, n_nodes - 1)] - g_lo,
                      -1.0).astype(np.float32)
        n_graphs = int(bl[bl >= 0].max()) + 1 if (bl >= 0).any() else 0
        assert n_graphs <= P, n_graphs
        batch_loc = bl.reshape(bpc, P).T.copy()                 # [P, bpc]
        cores.append(dict(idx=np.ascontiguousarray(idx_stream),
                          dstl=np.ascontiguousarray(dst_stream),
                          bloc=np.ascontiguousarray(batch_loc),
                          g_lo=g_lo, n_graphs=n_graphs))
    all_perm = perm_ids(np.arange(npad))
    return cores, k_lo, k_hi, bpc, single_win, hi_start, all_perm, qstarts


# ---------------------------------------------------------------- program
def _build(npad, bpc, k_lo, k_hi, single_win, hi_start):
    kt_b = [int(k_lo[b] + k_hi[b]) for b in range(bpc)]
    koff = np.concatenate([[0], np.cumsum(kt_b)]).astype(int)  # chunk offsets
    kt = int(koff[-1])  # total chunks per core
    npc = bpc * P
    f32 = mybir.dt.float32
    nc = bacc.Bacc("TRN2", target_bir_lowering=False, debug=False,
                   num_devices=NCORES)

    x_full = nc.declare_dram_parameter("x_full", [npad, D], f32, isOutput=False)
    xT_own = nc.declare_dram_parameter("xT_own", [P, npc], f32, isOutput=False)
    idx_in = nc.declare_dram_parameter("idx", [P, kt * 8], mybir.dt.int16,
                                       isOutput=False)
    dst_in = nc.declare_dram_parameter("dstl", [P, kt], f32, isOutput=False)
    bloc_in = nc.declare_dram_parameter("bloc", [P, bpc], f32, isOutput=False)
    w1_in = nc.declare_dram_parameter("W1", [L, D, D], f32, isOutput=False)
    w2_in = nc.declare_dram_parameter("W2", [L, D, D], f32, isOutput=False)
    b1_in = nc.declare_dram_parameter("b1T", [P, L], f32, isOutput=False)
    b2_in = nc.declare_dram_parameter("b2T", [P, L], f32, isOutput=False)
    iota_in = nc.declare_dram_parameter("iota", [P, P], f32, isOutput=False)
    y_out = [nc.declare_dram_parameter(f"y{l}", [npc, D], f32, isOutput=True)
             for l in range(L)]
    pool_out = [nc.declare_dram_parameter(f"pool{l}", [P, D], f32, isOutput=True)
                for l in range(L)]

    relu = mybir.ActivationFunctionType.Relu
    L_RUN = 1 if os.environ.get("KV_ONE_LAYER") else L
    SKIP_COLL = bool(os.environ.get("KV_SKIP_COLL"))
    SKIP_TR = bool(os.environ.get("KV_SKIP_TR"))
    POOL_NOACC = bool(os.environ.get("KV_POOL_NOACC"))

    with tile.TileContext(nc) as tc, ExitStack() as ctx:
        cpool = ctx.enter_context(tc.tile_pool(name="const", bufs=1))
        hpool = ctx.enter_context(tc.tile_pool(name="h", bufs=1))
        mpool = ctx.enter_context(tc.tile_pool(name="msgs", bufs=3))
        epool = ctx.enter_context(tc.tile_pool(name="emat", bufs=3))
        zpool = ctx.enter_context(tc.tile_pool(name="z", bufs=3))
        agg_pp = ctx.enter_context(tc.tile_pool(name="aggp", bufs=2, space="PSUM"))
        mlp_pp = ctx.enter_context(tc.tile_pool(name="mlpp", bufs=1, space="PSUM"))
        tr_pp = ctx.enter_context(tc.tile_pool(name="trp", bufs=1, space="PSUM"))
        pool_pp = ctx.enter_context(tc.tile_pool(name="poolp", bufs=1, space="PSUM"))
        dpool = ctx.enter_context(tc.tile_pool(name="dram", bufs=1, space="DRAM"))

        idx_sb = cpool.tile([P, kt * 8], mybir.dt.int16)
        nc.sync.dma_start(idx_sb[:], idx_in[:])
        dst_sb = cpool.tile([P, kt], f32)
        nc.sync.dma_start(dst_sb[:], dst_in[:])
        bloc_sb = cpool.tile([P, bpc], f32)
        nc.sync.dma_start(bloc_sb[:], bloc_in[:])
        iota_sb = cpool.tile([P, P], f32)
        nc.sync.dma_start(iota_sb[:], iota_in[:])
        ident_sb = cpool.tile([P, P], f32)
        make_identity(nc, ident_sb[:])
        w1_sb = []
        w2_sb = []
        for l in range(L):
            wt1 = cpool.tile([P, D], f32, tag=f"w1_{l}", name=f"w1_{l}")
            wt2 = cpool.tile([P, D], f32, tag=f"w2_{l}", name=f"w2_{l}")
            w1_sb.append(wt1)
            w2_sb.append(wt2)
        for l in range(L):
            nc.sync.dma_start(w1_sb[l][:], w1_in[l])
            nc.sync.dma_start(w2_sb[l][:], w2_in[l])
        b1_sb = cpool.tile([P, L], f32)
        nc.sync.dma_start(b1_sb[:], b1_in[:])
        b2_sb = cpool.tile([P, L], f32)
        nc.sync.dma_start(b2_sb[:], b2_in[:])

        hT_a = hpool.tile([P, npc], f32, tag="hta")
        hT_b = hpool.tile([P, npc], f32, tag="htb")
        hT = [hT_a, hT_b]
        nc.sync.dma_start(hT[0][:], xT_own[:])

        hown = []
        hfull = []
        for l in range(L - 1):
            ho = dpool.tile([npc, D], f32, tag=f"hown{l}", name=f"hown{l}")
            hf_t = dpool.tile([npad, D], f32, tag=f"hfull{l}", name=f"hfull{l}")
            hown.append(ho)
            hfull.append(hf_t)

        for l in range(L_RUN):
            hT_cur, hT_nxt = hT[l % 2], hT[(l + 1) % 2]
            pooled_ps = pool_pp.tile([P, D], f32, space="PSUM", tag="pool")
            for b in range(bpc):
                # ---- gather messages for this block's edges
                if l == 0:
                    src_lo = x_full[0:min(WIN, npad), :]
                    src_hi = None if single_win else x_full[hi_start:npad, :]
                else:
                    hf = hfull[l - 1]
                    src_lo = hf[0:min(WIN, npad), :]
                    src_hi = None if single_win else hf[hi_start:npad, :]
                klo_b, khi_b = int(k_lo[b]), int(k_hi[b])
                ktb = kt_b[b]
                seg8 = int(koff[b]) * 8
                msgs_lo = mpool.tile([P, klo_b, D], f32, tag="mlo",
                                     padded_shape=[P, int(k_lo.max()), D])
                nc.gpsimd.dma_gather(
                    out_ap=msgs_lo[:], in_ap=src_lo,
                    idxs_ap=idx_sb[:, seg8:seg8 + klo_b * 8],
                    num_idxs=klo_b * P, num_idxs_reg=klo_b * P, elem_size=D,
                    single_packet=False)
                if khi_b:
                    msgs_hi = mpool.tile([P, khi_b, D], f32, tag="mhi",
                                         padded_shape=[P, int(k_hi.max()), D])
                    nc.gpsimd.dma_gather(
                        out_ap=msgs_hi[:], in_ap=src_hi,
                        idxs_ap=idx_sb[:, seg8 + klo_b * 8:seg8 + ktb * 8],
                        num_idxs=khi_b * P, num_idxs_reg=khi_b * P, elem_size=D,
                        single_packet=False)

                # ---- segment-sum: aggT[d, n] += sum_j msgs[j, d] * Emat[j, n]
                agg_ps = agg_pp.tile([P, P], f32, space="PSUM", tag="agg")
                for k in range(ktb):
                    col = int(koff[b]) + k
                    emat = epool.tile([P, P], f32, tag="emat")
                    nc.vector.tensor_tensor(
                        out=emat[:],
                        in0=dst_sb[:, col:col + 1].to_broadcast([P, P])[:],
                        in1=iota_sb[:],
                        op=mybir.AluOpType.is_equal)
                    rhs = (msgs_lo[:, k, :] if k < klo_b
                           else msgs_hi[:, k - klo_b, :])
                    nc.tensor.matmul(out=agg_ps[:], lhsT=rhs, rhs=emat[:],
                                     start=(k == 0), stop=(k == ktb - 1))

                # ---- zT = hT + aggT ; MLP (transposed); relu
                zT = zpool.tile([P, P], f32, tag="zT")
                nc.vector.tensor_add(zT[:], agg_ps[:],
                                     hT_cur[:, b * P:(b + 1) * P])
                mm1 = mlp_pp.tile([P, P], f32, space="PSUM", tag="mm1")
                nc.tensor.matmul(out=mm1[:], lhsT=w1_sb[l][:], rhs=zT[:],
                                 start=True, stop=True)
                a1T = zpool.tile([P, P], f32, tag="a1T")
                nc.scalar.activation(a1T[:], mm1[:], relu, bias=b1_sb[:, l:l + 1])
                mm2 = mlp_pp.tile([P, P], f32, space="PSUM", tag="mm2")
                nc.tensor.matmul(out=mm2[:], lhsT=w2_sb[l][:], rhs=a1T[:],
                                 start=True, stop=True)
                nc.scalar.activation(hT_nxt[:, b * P:(b + 1) * P], mm2[:], relu,
                                     bias=b2_sb[:, l:l + 1])

                # ---- natural-layout h block (for DRAM rows + pooling)
                hnew = zpool.tile([P, P], f32, tag="hnew")
                if SKIP_TR:
                    nc.vector.tensor_copy(hnew[:], hT_nxt[:, b * P:(b + 1) * P])
                else:
                    tr_ps = tr_pp.tile([P, P], f32, space="PSUM", tag="tr")
                    nc.tensor.transpose(out=tr_ps[:],
                                        in_=hT_nxt[:, b * P:(b + 1) * P],
                                        identity=ident_sb[:])
                    nc.vector.tensor_copy(hnew[:], tr_ps[:])
                nc.sync.dma_start(y_out[l][b * P:(b + 1) * P, :], hnew[:])
                if l < L - 1:
                    nc.sync.dma_start(hown[l][b * P:(b + 1) * P, :], hnew[:])

                # ---- pooling: pooled[g, d] += sum_n Bmat[n, g] * hnew[n, d]
                qsz = (bpc + 3) // 4
                if (l < L_RUN - 1 and not SKIP_COLL
                        and ((b + 1) % qsz == 0 or b == bpc - 1)):
                    r0 = (b // qsz) * qsz * P
                    r1 = min((b + 1) * P, npc)
                    nc.gpsimd.collective_compute(
                        "AllGather", mybir.AluOpType.bypass,
                        replica_groups=[list(range(NCORES))],
                        ins=[hown[l][r0:r1, :]],
                        outs=[hfull[l][NCORES * r0:NCORES * r1, :]])
                bmat = epool.tile([P, P], f32, tag="bmat")
                nc.vector.tensor_tensor(
                    out=bmat[:],
                    in0=bloc_sb[:, b:b + 1].to_broadcast([P, P])[:],
                    in1=iota_sb[:],
                    op=mybir.AluOpType.is_equal)
                nc.tensor.matmul(out=pooled_ps[:], lhsT=bmat[:], rhs=hnew[:],
                                 start=(b == 0) or POOL_NOACC,
                                 stop=(b == bpc - 1) or POOL_NOACC)

            pooled_sb = zpool.tile([P, D], f32, tag="pooled")
            nc.vector.tensor_copy(pooled_sb[:], pooled_ps[:])
            nc.sync.dma_start(pool_out[l][:], pooled_sb[:])

            if l < L_RUN - 1:
                if SKIP_COLL:
                    nc.sync.dma_start(hfull[l][0:npc, :], hown[l][:])
                else:
                    pass  # chunked allgather issued inside the block loop

    nc.finalize()
    return nc


# ---------------------------------------------------------------- entry
def kernel(x, edge_index, batch, W1, b1, W2, b2):
    global LAST_RESULT
    x = np.asarray(x, np.float32)
    edge_index = np.asarray(edge_index, np.int64)
    batch = np.asarray(batch, np.int64)
    W1 = np.ascontiguousarray(np.asarray(W1, np.float32))
    W2 = np.ascontiguousarray(np.asarray(W2, np.float32))
    b1 = np.asarray(b1, np.float32)
    b2 = np.asarray(b2, np.float32)

    n, d = x.shape
    assert d == D
    num_graphs = int(batch.max()) + 1
    npad = ((n + NCORES * P - 1) // (NCORES * P)) * (NCORES * P)
    npc = npad // NCORES

    src, dst = edge_index[0], edge_index[1]
    cores, k_lo, k_hi, bpc, single_win, hi_start, all_perm, qstarts = _prep(
        src, dst, batch, n, npad)

    nc = _build(npad, bpc, k_lo, k_hi, single_win, hi_start)

    x_pad = np.zeros((npad, D), np.float32)
    x_pad[:n] = x
    x_perm = np.zeros((npad, D), np.float32)
    x_perm[all_perm] = x_pad
    iota = np.tile(np.arange(P, dtype=np.float32), (P, 1))
    b1T = np.tile(b1.T[:, None, :], (1, 1, 1)).reshape(D, L)  # [D, L]
    b1T = np.ascontiguousarray(b1.T)                          # [D, L]
    b2T = np.ascontiguousarray(b2.T)

    in_maps = []
    for c in range(NCORES):
        cc = cores[c]
        xT = np.ascontiguousarray(x_pad[c * npc:(c + 1) * npc].T)
        in_maps.append(dict(x_full=x_perm, xT_own=xT, idx=cc['idx'],
                            dstl=cc['dstl'], bloc=cc['bloc'],
                            W1=W1, W2=W2, b1T=b1T, b2T=b2T, iota=iota))

    res = run_bass_kernel_spmd(nc, in_maps, list(range(NCORES)),
                               trace=bool(os.environ.get("BASS_TRACE")))
    LAST_RESULT = res

    xs = np.empty((L, n, D), np.float32)
    pooled = np.zeros((L, num_graphs, D), np.float32)
    for c in range(NCORES):
        r = res.results[c]
        lo, hi = c * npc, min((c + 1) * npc, n)
        for l in range(L):
            xs[l, lo:hi] = r[f"y{l}"][:hi - lo]
            g = cores[c]
            pooled[l, g['g_lo']:g['g_lo'] + g['n_graphs']] += \
                r[f"pool{l}"][:g['n_graphs']]
    xs_cat = np.concatenate(list(xs), axis=1)
    pooled_cat = np.concatenate(list(pooled), axis=1)
    return pooled_cat, xs_cat


# revision 10
# speedup vs baseline: 2.0124x; 1.6783x over previous
"""GIN encoder (3-layer) on 8 Trainium2 NeuronCores.

Sharding: nodes partitioned into 8 contiguous ranges (by dst); each core
processes the edges targeting its nodes.  Per layer:
  - per-edge messages h[src] gathered from a full replica of h in local HBM
    via gpsimd dma_gather (int16 indices -> two overlapping 32768-row windows)
  - segment-sum by dst via one-hot matmuls accumulated in PSUM
    (out = msgs.T @ E_mat gives transposed aggregates)
  - MLP with stationary weights: W.T @ zT, per-partition bias+ReLU on ACT
  - per-graph pooling via one-hot matmul (batch is sorted)
  - new h blocks pushed to outputs; AllGather rebuilds the full replica
    between layers.
"""
import os
import sys

sys.path.insert(0, '/opt/trn_rl_repo')

import numpy as np
from contextlib import ExitStack

from concourse import bacc, mybir, tile
from concourse.bass_utils import run_bass_kernel_spmd
from concourse.masks import make_identity

P = 128
D = 128
NCORES = 8
L = 3
WIN = 32768  # int16-addressable window rows

LAST_RESULT = None  # test harness can read exec_time_ns from here


# ---------------------------------------------------------------- host prep
def _prep(src, dst, batch, n_nodes, npad):
    """Partition edges by dst range, build padded per-block chunk streams."""
    ncore_nodes = npad // NCORES
    bpc = ncore_nodes // P
    nblk = NCORES * bpc
    single_win = npad <= WIN
    hi_start = 0 if single_win else npad - WIN

    # permute node ids so each per-quarter AllGather output is contiguous:
    # new id = [quarter][core][row within quarter]
    qn = (bpc + 3) // 4
    qstarts = np.array([min(i * qn * P, ncore_nodes) for i in range(5)])
    qlens = np.diff(qstarts)
    qbase8 = np.concatenate([[0], np.cumsum(qlens * NCORES)])

    def perm_ids(r):
        c, local = r // ncore_nodes, r % ncore_nodes
        q = np.searchsorted(qstarts, local, side='right') - 1
        return qbase8[q] + c * qlens[q] + (local - qstarts[q])

    psrc = perm_ids(src)
    blk = dst // P                      # global block of each edge
    is_hi = np.zeros_like(src, dtype=bool) if single_win else (psrc >= WIN)
    key = blk * 2 + is_hi
    order = np.argsort(key, kind='stable')
    src_s, dst_s, key_s = psrc[order], dst[order], key[order]
    cnt = np.bincount(key_s, minlength=nblk * 2)
    offs = np.concatenate([[0], np.cumsum(cnt)])

    cnt_lo = cnt[0::2].reshape(NCORES, bpc)
    cnt_hi = cnt[1::2].reshape(NCORES, bpc)
    k_lo = np.maximum(1, np.ceil(cnt_lo.max(axis=0) / P).astype(int))  # [bpc]
    k_hi = (np.zeros(bpc, int) if single_win
            else np.ceil(cnt_hi.max(axis=0) / P).astype(int))

    cores = []
    for c in range(NCORES):
        idx_segs, dst_cols = [], []
        for b in range(bpc):
            g = c * bpc + b
            blk_start = g * P
            for half, k in ((0, int(k_lo[b])), (1, int(k_hi[b]))):
                if k == 0:
                    continue
                e0, e1 = offs[2 * g + half], offs[2 * g + half + 1]
                n = e1 - e0
                cap = k * P
                idx = np.zeros(cap, np.int16)
                dl = np.full(cap, -1.0, np.float32)
                s = src_s[e0:e1]
                idx[:n] = (s if half == 0 else s - hi_start).astype(np.int16)
                dl[:n] = (dst_s[e0:e1] - blk_start).astype(np.float32)
                # wrap this segment: position i -> [i%16, i//16]
                idx_segs.append(idx.reshape(-1, 16).T)         # [16, cap/16]
                dst_cols.append(dl.reshape(-1, P).T)            # [P, k]
        idx_stream = np.tile(np.concatenate(idx_segs, axis=1), (8, 1))
        dst_stream = np.concatenate(dst_cols, axis=1)
        # batch_local per block column; pad nodes (>= n_nodes) -> -1
        nodes = c * ncore_nodes + np.arange(ncore_nodes)
        g_lo = int(batch[min(c * ncore_nodes, n_nodes - 1)])
        bl = np.where(nodes < n_nodes, batch[np.minimum(nodes, n_nodes - 1)] - g_lo,
                      -1.0).astype(np.float32)
        n_graphs = int(bl[bl >= 0].max()) + 1 if (bl >= 0).any() else 0
        assert n_graphs <= P, n_graphs
        batch_loc = bl.reshape(bpc, P).T.copy()                 # [P, bpc]
        cores.append(dict(idx=np.ascontiguousarray(idx_stream),
                          dstl=np.ascontiguousarray(dst_stream),
                          bloc=np.ascontiguousarray(batch_loc),
                          g_lo=g_lo, n_graphs=n_graphs))
    all_perm = perm_ids(np.arange(npad))
    return cores, k_lo, k_hi, bpc, single_win, hi_start, all_perm, qstarts


# ---------------------------------------------------------------- program
def _build(npad, bpc, k_lo, k_hi, single_win, hi_start):
    kt_b = [int(k_lo[b] + k_hi[b]) for b in range(bpc)]
    koff = np.concatenate([[0], np.cumsum(kt_b)]).astype(int)  # chunk offsets
    kt = int(koff[-1])  # total chunks per core
    npc = bpc * P
    f32 = mybir.dt.float32
    nc = bacc.Bacc("TRN2", target_bir_lowering=False, debug=False,
                   num_devices=NCORES, num_swdge_queues=4)

    x_full = nc.declare_dram_parameter("x_full", [npad, D], f32, isOutput=False)
    xT_own = nc.declare_dram_parameter("xT_own", [P, npc], f32, isOutput=False)
    idx_in = nc.declare_dram_parameter("idx", [P, kt * 8], mybir.dt.int16,
                                       isOutput=False)
    dst_in = nc.declare_dram_parameter("dstl", [P, kt], f32, isOutput=False)
    bloc_in = nc.declare_dram_parameter("bloc", [P, bpc], f32, isOutput=False)
    w1_in = nc.declare_dram_parameter("W1", [L, D, D], f32, isOutput=False)
    w2_in = nc.declare_dram_parameter("W2", [L, D, D], f32, isOutput=False)
    b1_in = nc.declare_dram_parameter("b1T", [P, L], f32, isOutput=False)
    b2_in = nc.declare_dram_parameter("b2T", [P, L], f32, isOutput=False)
    iota_in = nc.declare_dram_parameter("iota", [P, P], f32, isOutput=False)
    y_out = [nc.declare_dram_parameter(f"y{l}", [npc, D], f32, isOutput=True)
             for l in range(L)]
    pool_out = [nc.declare_dram_parameter(f"pool{l}", [P, D], f32, isOutput=True)
                for l in range(L)]

    relu = mybir.ActivationFunctionType.Relu
    L_RUN = 1 if os.environ.get("KV_ONE_LAYER") else L
    SKIP_COLL = bool(os.environ.get("KV_SKIP_COLL"))
    SKIP_TR = bool(os.environ.get("KV_SKIP_TR"))
    POOL_NOACC = bool(os.environ.get("KV_POOL_NOACC"))

    with tile.TileContext(nc) as tc, ExitStack() as ctx:
        cpool = ctx.enter_context(tc.tile_pool(name="const", bufs=1))
        hpool = ctx.enter_context(tc.tile_pool(name="h", bufs=1))
        mpool = ctx.enter_context(tc.tile_pool(name="msgs", bufs=3))
        epool = ctx.enter_context(tc.tile_pool(name="emat", bufs=3))
        zpool = ctx.enter_context(tc.tile_pool(name="z", bufs=3))
        agg_pp = ctx.enter_context(tc.tile_pool(name="aggp", bufs=2, space="PSUM"))
        mlp_pp = ctx.enter_context(tc.tile_pool(name="mlpp", bufs=1, space="PSUM"))
        tr_pp = ctx.enter_context(tc.tile_pool(name="trp", bufs=1, space="PSUM"))
        pool_pp = ctx.enter_context(tc.tile_pool(name="poolp", bufs=1, space="PSUM"))
        dpool = ctx.enter_context(tc.tile_pool(name="dram", bufs=1, space="DRAM"))

        idx_sb = cpool.tile([P, kt * 8], mybir.dt.int16)
        nc.sync.dma_start(idx_sb[:], idx_in[:])
        dst_sb = cpool.tile([P, kt], f32)
        nc.sync.dma_start(dst_sb[:], dst_in[:])
        bloc_sb = cpool.tile([P, bpc], f32)
        nc.sync.dma_start(bloc_sb[:], bloc_in[:])
        iota_sb = cpool.tile([P, P], f32)
        nc.sync.dma_start(iota_sb[:], iota_in[:])
        ident_sb = cpool.tile([P, P], f32)
        make_identity(nc, ident_sb[:])
        w1_sb = []
        w2_sb = []
        for l in range(L):
            wt1 = cpool.tile([P, D], f32, tag=f"w1_{l}", name=f"w1_{l}")
            wt2 = cpool.tile([P, D], f32, tag=f"w2_{l}", name=f"w2_{l}")
            w1_sb.append(wt1)
            w2_sb.append(wt2)
        for l in range(L):
            nc.sync.dma_start(w1_sb[l][:], w1_in[l])
            nc.sync.dma_start(w2_sb[l][:], w2_in[l])
        b1_sb = cpool.tile([P, L], f32)
        nc.sync.dma_start(b1_sb[:], b1_in[:])
        b2_sb = cpool.tile([P, L], f32)
        nc.sync.dma_start(b2_sb[:], b2_in[:])

        hT_a = hpool.tile([P, npc], f32, tag="hta")
        hT_b = hpool.tile([P, npc], f32, tag="htb")
        hT = [hT_a, hT_b]
        nc.sync.dma_start(hT[0][:], xT_own[:])

        hown = []
        hfull = []
        for l in range(L - 1):
            ho = dpool.tile([npc, D], f32, tag=f"hown{l}", name=f"hown{l}")
            hf_t = dpool.tile([npad, D], f32, tag=f"hfull{l}", name=f"hfull{l}")
            hown.append(ho)
            hfull.append(hf_t)

        for l in range(L_RUN):
            hT_cur, hT_nxt = hT[l % 2], hT[(l + 1) % 2]
            pooled_ps = pool_pp.tile([P, D], f32, space="PSUM", tag="pool")
            for b in range(bpc):
                # ---- gather messages for this block's edges
                if l == 0:
                    src_lo = x_full[0:min(WIN, npad), :]
                    src_hi = None if single_win else x_full[hi_start:npad, :]
                else:
                    hf = hfull[l - 1]
                    src_lo = hf[0:min(WIN, npad), :]
                    src_hi = None if single_win else hf[hi_start:npad, :]
                klo_b, khi_b = int(k_lo[b]), int(k_hi[b])
                ktb = kt_b[b]
                seg8 = int(koff[b]) * 8
                msgs_lo = mpool.tile([P, klo_b, D], f32, tag="mlo",
                                     padded_shape=[P, int(k_lo.max()), D])
                nc.gpsimd.dma_gather(
                    out_ap=msgs_lo[:], in_ap=src_lo,
                    idxs_ap=idx_sb[:, seg8:seg8 + klo_b * 8],
                    num_idxs=klo_b * P, num_idxs_reg=klo_b * P, elem_size=D,
                    single_packet=False, queue_num=(2 * b) % 4)
                if khi_b:
                    msgs_hi = mpool.tile([P, khi_b, D], f32, tag="mhi",
                                         padded_shape=[P, int(k_hi.max()), D])
                    nc.gpsimd.dma_gather(
                        out_ap=msgs_hi[:], in_ap=src_hi,
                        idxs_ap=idx_sb[:, seg8 + klo_b * 8:seg8 + ktb * 8],
                        num_idxs=khi_b * P, num_idxs_reg=khi_b * P, elem_size=D,
                        single_packet=False, queue_num=(2 * b + 1) % 4)

                # ---- segment-sum: aggT[d, n] += sum_j msgs[j, d] * Emat[j, n]
                agg_ps = agg_pp.tile([P, P], f32, space="PSUM", tag="agg")
                for k in range(ktb):
                    col = int(koff[b]) + k
                    emat = epool.tile([P, P], f32, tag="emat")
                    nc.vector.tensor_tensor(
                        out=emat[:],
                        in0=dst_sb[:, col:col + 1].to_broadcast([P, P])[:],
                        in1=iota_sb[:],
                        op=mybir.AluOpType.is_equal)
                    rhs = (msgs_lo[:, k, :] if k < klo_b
                           else msgs_hi[:, k - klo_b, :])
                    nc.tensor.matmul(out=agg_ps[:], lhsT=rhs, rhs=emat[:],
                                     start=(k == 0), stop=(k == ktb - 1))

                # ---- zT = hT + aggT ; MLP (transposed); relu
                zT = zpool.tile([P, P], f32, tag="zT")
                nc.vector.tensor_add(zT[:], agg_ps[:],
                                     hT_cur[:, b * P:(b + 1) * P])
                mm1 = mlp_pp.tile([P, P], f32, space="PSUM", tag="mm1")
                nc.tensor.matmul(out=mm1[:], lhsT=w1_sb[l][:], rhs=zT[:],
                                 start=True, stop=True)
                a1T = zpool.tile([P, P], f32, tag="a1T")
                nc.scalar.activation(a1T[:], mm1[:], relu, bias=b1_sb[:, l:l + 1])
                mm2 = mlp_pp.tile([P, P], f32, space="PSUM", tag="mm2")
                nc.tensor.matmul(out=mm2[:], lhsT=w2_sb[l][:], rhs=a1T[:],
                                 start=True, stop=True)
                nc.scalar.activation(hT_nxt[:, b * P:(b + 1) * P], mm2[:], relu,
                                     bias=b2_sb[:, l:l + 1])

                # ---- natural-layout h block (for DRAM rows + pooling)
                hnew = zpool.tile([P, P], f32, tag="hnew")
                if SKIP_TR:
                    nc.vector.tensor_copy(hnew[:], hT_nxt[:, b * P:(b + 1) * P])
                else:
                    tr_ps = tr_pp.tile([P, P], f32, space="PSUM", tag="tr")
                    nc.tensor.transpose(out=tr_ps[:],
                                        in_=hT_nxt[:, b * P:(b + 1) * P],
                                        identity=ident_sb[:])
                    nc.vector.tensor_copy(hnew[:], tr_ps[:])
                nc.sync.dma_start(y_out[l][b * P:(b + 1) * P, :], hnew[:])
                if l < L - 1:
                    nc.sync.dma_start(hown[l][b * P:(b + 1) * P, :], hnew[:])

                # ---- pooling: pooled[g, d] += sum_n Bmat[n, g] * hnew[n, d]
                qsz = (bpc + 3) // 4
                if (l < L_RUN - 1 and not SKIP_COLL
                        and ((b + 1) % qsz == 0 or b == bpc - 1)):
                    r0 = (b // qsz) * qsz * P
                    r1 = min((b + 1) * P, npc)
                    nc.gpsimd.collective_compute(
                        "AllGather", mybir.AluOpType.bypass,
                        replica_groups=[list(range(NCORES))],
                        ins=[hown[l][r0:r1, :]],
                        outs=[hfull[l][NCORES * r0:NCORES * r1, :]])
                bmat = epool.tile([P, P], f32, tag="bmat")
                nc.vector.tensor_tensor(
                    out=bmat[:],
                    in0=bloc_sb[:, b:b + 1].to_broadcast([P, P])[:],
                    in1=iota_sb[:],
                    op=mybir.AluOpType.is_equal)
                nc.tensor.matmul(out=pooled_ps[:], lhsT=bmat[:], rhs=hnew[:],
                                 start=(b == 0) or POOL_NOACC,
                                 stop=(b == bpc - 1) or POOL_NOACC)

            pooled_sb = zpool.tile([P, D], f32, tag="pooled")
            nc.vector.tensor_copy(pooled_sb[:], pooled_ps[:])
            nc.sync.dma_start(pool_out[l][:], pooled_sb[:])

            if l < L_RUN - 1:
                if SKIP_COLL:
                    nc.sync.dma_start(hfull[l][0:npc, :], hown[l][:])
                else:
                    pass  # chunked allgather issued inside the block loop

    nc.finalize()
    return nc


# ---------------------------------------------------------------- entry
def kernel(x, edge_index, batch, W1, b1, W2, b2):
    global LAST_RESULT
    x = np.asarray(x, np.float32)
    edge_index = np.asarray(edge_index, np.int64)
    batch = np.asarray(batch, np.int64)
    W1 = np.ascontiguousarray(np.asarray(W1, np.float32))
    W2 = np.ascontiguousarray(np.asarray(W2, np.float32))
    b1 = np.asarray(b1, np.float32)
    b2 = np.asarray(b2, np.float32)

    n, d = x.shape
    assert d == D
    num_graphs = int(batch.max()) + 1
    npad = ((n + NCORES * P - 1) // (NCORES * P)) * (NCORES * P)
    npc = npad // NCORES

    src, dst = edge_index[0], edge_index[1]
    cores, k_lo, k_hi, bpc, single_win, hi_start, all_perm, qstarts = _prep(
        src, dst, batch, n, npad)

    nc = _build(npad, bpc, k_lo, k_hi, single_win, hi_start)

    x_pad = np.zeros((npad, D), np.float32)
    x_pad[:n] = x
    x_perm = np.zeros((npad, D), np.float32)
    x_perm[all_perm] = x_pad
    iota = np.tile(np.arange(P, dtype=np.float32), (P, 1))
    b1T = np.tile(b1.T[:, None, :], (1, 1, 1)).reshape(D, L)  # [D, L]
    b1T = np.ascontiguousarray(b1.T)                          # [D, L]
    b2T = np.ascontiguousarray(b2.T)

    in_maps = []
    for c in range(NCORES):
        cc = cores[c]
        xT = np.ascontiguousarray(x_pad[c * npc:(c + 1) * npc].T)
        in_maps.append(dict(x_full=x_perm, xT_own=xT, idx=cc['idx'],
                            dstl=cc['dstl'], bloc=cc['bloc'],
                            W1=W1, W2=W2, b1T=b1T, b2T=b2T, iota=iota))

    res = run_bass_kernel_spmd(nc, in_maps, list(range(NCORES)),
                               trace=bool(os.environ.get("BASS_TRACE")))
    LAST_RESULT = res

    xs = np.empty((L, n, D), np.float32)
    pooled = np.zeros((L, num_graphs, D), np.float32)
    for c in range(NCORES):
        r = res.results[c]
        lo, hi = c * npc, min((c + 1) * npc, n)
        for l in range(L):
            xs[l, lo:hi] = r[f"y{l}"][:hi - lo]
            g = cores[c]
            pooled[l, g['g_lo']:g['g_lo'] + g['n_graphs']] += \
                r[f"pool{l}"][:g['n_graphs']]
    xs_cat = np.concatenate(list(xs), axis=1)
    pooled_cat = np.concatenate(list(pooled), axis=1)
    return pooled_cat, xs_cat


# revision 11
# speedup vs baseline: 2.1881x; 1.0873x over previous
"""GIN encoder (3-layer) on 8 Trainium2 NeuronCores.

Sharding: nodes partitioned into 8 contiguous ranges (by dst); each core
processes the edges targeting its nodes.  Per layer:
  - per-edge messages h[src] gathered from a full replica of h in local HBM
    via gpsimd dma_gather (int16 indices -> two overlapping 32768-row windows)
  - segment-sum by dst via one-hot matmuls accumulated in PSUM
    (out = msgs.T @ E_mat gives transposed aggregates)
  - MLP with stationary weights: W.T @ zT, per-partition bias+ReLU on ACT
  - per-graph pooling via one-hot matmul (batch is sorted)
  - new h blocks pushed to outputs; AllGather rebuilds the full replica
    between layers.
"""
import os
import sys

sys.path.insert(0, '/opt/trn_rl_repo')

import numpy as np
from contextlib import ExitStack

from concourse import bacc, mybir, tile
from concourse.bass_utils import run_bass_kernel_spmd
from concourse.masks import make_identity

P = 128
D = 128
NCORES = 8
L = 3
WIN = 32768  # int16-addressable window rows

LAST_RESULT = None  # test harness can read exec_time_ns from here


# ---------------------------------------------------------------- host prep
def _prep(src, dst, batch, n_nodes, npad):
    """Partition edges by dst range, build padded per-block chunk streams."""
    ncore_nodes = npad // NCORES
    bpc = ncore_nodes // P
    nblk = NCORES * bpc
    single_win = npad <= WIN
    hi_start = 0 if single_win else npad - WIN

    # permute node ids so each per-quarter AllGather output is contiguous:
    # new id = [quarter][core][row within quarter]
    qn = (bpc + 3) // 4
    qstarts = np.array([min(i * qn * P, ncore_nodes) for i in range(5)])
    qlens = np.diff(qstarts)
    qbase8 = np.concatenate([[0], np.cumsum(qlens * NCORES)])

    def perm_ids(r):
        c, local = r // ncore_nodes, r % ncore_nodes
        q = np.searchsorted(qstarts, local, side='right') - 1
        return qbase8[q] + c * qlens[q] + (local - qstarts[q])

    psrc = perm_ids(src)
    blk = dst // P                      # global block of each edge
    is_hi = np.zeros_like(src, dtype=bool) if single_win else (psrc >= WIN)
    key = blk * 2 + is_hi
    order = np.argsort(key, kind='stable')
    src_s, dst_s, key_s = psrc[order], dst[order], key[order]
    cnt = np.bincount(key_s, minlength=nblk * 2)
    offs = np.concatenate([[0], np.cumsum(cnt)])

    cnt_lo = cnt[0::2].reshape(NCORES, bpc)
    cnt_hi = cnt[1::2].reshape(NCORES, bpc)
    k_lo = np.maximum(1, np.ceil(cnt_lo.max(axis=0) / P).astype(int))  # [bpc]
    k_hi = (np.zeros(bpc, int) if single_win
            else np.ceil(cnt_hi.max(axis=0) / P).astype(int))

    cores = []
    for c in range(NCORES):
        idx_segs, dst_cols = [], []
        for b in range(bpc):
            g = c * bpc + b
            blk_start = g * P
            for half, k in ((0, int(k_lo[b])), (1, int(k_hi[b]))):
                if k == 0:
                    continue
                e0, e1 = offs[2 * g + half], offs[2 * g + half + 1]
                n = e1 - e0
                cap = k * P
                idx = np.zeros(cap, np.int16)
                dl = np.full(cap, -1.0, np.float32)
                s = src_s[e0:e1]
                idx[:n] = (s if half == 0 else s - hi_start).astype(np.int16)
                dl[:n] = (dst_s[e0:e1] - blk_start).astype(np.float32)
                # wrap this segment: position i -> [i%16, i//16]
                idx_segs.append(idx.reshape(-1, 16).T)         # [16, cap/16]
                dst_cols.append(dl.reshape(-1, P).T)            # [P, k]
        idx_stream = np.tile(np.concatenate(idx_segs, axis=1), (8, 1))
        dst_stream = np.concatenate(dst_cols, axis=1)
        # batch_local per block column; pad nodes (>= n_nodes) -> -1
        nodes = c * ncore_nodes + np.arange(ncore_nodes)
        g_lo = int(batch[min(c * ncore_nodes, n_nodes - 1)])
        bl = np.where(nodes < n_nodes, batch[np.minimum(nodes, n_nodes - 1)] - g_lo,
                      -1.0).astype(np.float32)
        n_graphs = int(bl[bl >= 0].max()) + 1 if (bl >= 0).any() else 0
        assert n_graphs <= P, n_graphs
        batch_loc = bl.reshape(bpc, P).T.copy()                 # [P, bpc]
        cores.append(dict(idx=np.ascontiguousarray(idx_stream),
                          dstl=np.ascontiguousarray(dst_stream),
                          bloc=np.ascontiguousarray(batch_loc),
                          g_lo=g_lo, n_graphs=n_graphs))
    all_perm = perm_ids(np.arange(npad))
    return cores, k_lo, k_hi, bpc, single_win, hi_start, all_perm, qstarts


# ---------------------------------------------------------------- program
def _build(npad, bpc, k_lo, k_hi, single_win, hi_start):
    kt_b = [int(k_lo[b] + k_hi[b]) for b in range(bpc)]
    koff = np.concatenate([[0], np.cumsum(kt_b)]).astype(int)  # chunk offsets
    kt = int(koff[-1])  # total chunks per core
    npc = bpc * P
    f32 = mybir.dt.float32
    nc = bacc.Bacc("TRN2", target_bir_lowering=False, debug=False,
                   num_devices=NCORES, num_swdge_queues=4)

    x_full = nc.declare_dram_parameter("x_full", [npad, D], f32, isOutput=False)
    xT_own = nc.declare_dram_parameter("xT_own", [P, npc], f32, isOutput=False)
    idx_in = nc.declare_dram_parameter("idx", [P, kt * 8], mybir.dt.int16,
                                       isOutput=False)
    dst_in = nc.declare_dram_parameter("dstl", [P, kt], f32, isOutput=False)
    bloc_in = nc.declare_dram_parameter("bloc", [P, bpc], f32, isOutput=False)
    w1_in = nc.declare_dram_parameter("W1", [L, D, D], f32, isOutput=False)
    w2_in = nc.declare_dram_parameter("W2", [L, D, D], f32, isOutput=False)
    b1_in = nc.declare_dram_parameter("b1T", [P, L], f32, isOutput=False)
    b2_in = nc.declare_dram_parameter("b2T", [P, L], f32, isOutput=False)
    iota_in = nc.declare_dram_parameter("iota", [P, P], f32, isOutput=False)
    y_out = [nc.declare_dram_parameter(f"y{l}", [npc, D], f32, isOutput=True)
             for l in range(L)]
    pool_out = [nc.declare_dram_parameter(f"pool{l}", [P, D], f32, isOutput=True)
                for l in range(L)]

    relu = mybir.ActivationFunctionType.Relu
    L_RUN = 1 if os.environ.get("KV_ONE_LAYER") else L
    SKIP_COLL = bool(os.environ.get("KV_SKIP_COLL"))
    SKIP_TR = bool(os.environ.get("KV_SKIP_TR"))
    POOL_NOACC = bool(os.environ.get("KV_POOL_NOACC"))

    with tile.TileContext(nc) as tc, ExitStack() as ctx:
        cpool = ctx.enter_context(tc.tile_pool(name="const", bufs=1))
        hpool = ctx.enter_context(tc.tile_pool(name="h", bufs=1))
        mpool = ctx.enter_context(tc.tile_pool(name="msgs", bufs=5))
        epool = ctx.enter_context(tc.tile_pool(name="emat", bufs=4))
        zpool = ctx.enter_context(tc.tile_pool(name="z", bufs=4))
        agg_pp = ctx.enter_context(tc.tile_pool(name="aggp", bufs=3, space="PSUM"))
        mlp_pp = ctx.enter_context(tc.tile_pool(name="mlpp", bufs=1, space="PSUM"))
        tr_pp = ctx.enter_context(tc.tile_pool(name="trp", bufs=1, space="PSUM"))
        pool_pp = ctx.enter_context(tc.tile_pool(name="poolp", bufs=1, space="PSUM"))
        dpool = ctx.enter_context(tc.tile_pool(name="dram", bufs=1, space="DRAM"))

        idx_sb = cpool.tile([P, kt * 8], mybir.dt.int16)
        nc.sync.dma_start(idx_sb[:], idx_in[:])
        dst_sb = cpool.tile([P, kt], f32)
        nc.sync.dma_start(dst_sb[:], dst_in[:])
        bloc_sb = cpool.tile([P, bpc], f32)
        nc.sync.dma_start(bloc_sb[:], bloc_in[:])
        iota_sb = cpool.tile([P, P], f32)
        nc.sync.dma_start(iota_sb[:], iota_in[:])
        ident_sb = cpool.tile([P, P], f32)
        make_identity(nc, ident_sb[:])
        w1_sb = []
        w2_sb = []
        for l in range(L):
            wt1 = cpool.tile([P, D], f32, tag=f"w1_{l}", name=f"w1_{l}")
            wt2 = cpool.tile([P, D], f32, tag=f"w2_{l}", name=f"w2_{l}")
            w1_sb.append(wt1)
            w2_sb.append(wt2)
        for l in range(L):
            nc.sync.dma_start(w1_sb[l][:], w1_in[l])
            nc.sync.dma_start(w2_sb[l][:], w2_in[l])
        b1_sb = cpool.tile([P, L], f32)
        nc.sync.dma_start(b1_sb[:], b1_in[:])
        b2_sb = cpool.tile([P, L], f32)
        nc.sync.dma_start(b2_sb[:], b2_in[:])

        hT_a = hpool.tile([P, npc], f32, tag="hta")
        hT_b = hpool.tile([P, npc], f32, tag="htb")
        hT = [hT_a, hT_b]
        nc.sync.dma_start(hT[0][:], xT_own[:])

        hown = []
        hfull = []
        for l in range(L - 1):
            ho = dpool.tile([npc, D], f32, tag=f"hown{l}", name=f"hown{l}")
            hf_t = dpool.tile([npad, D], f32, tag=f"hfull{l}", name=f"hfull{l}")
            hown.append(ho)
            hfull.append(hf_t)

        for l in range(L_RUN):
            hT_cur, hT_nxt = hT[l % 2], hT[(l + 1) % 2]
            pooled_ps = pool_pp.tile([P, D], f32, space="PSUM", tag="pool")
            for b in range(bpc):
                # ---- gather messages for this block's edges
                if l == 0:
                    src_lo = x_full[0:min(WIN, npad), :]
                    src_hi = None if single_win else x_full[hi_start:npad, :]
                else:
                    hf = hfull[l - 1]
                    src_lo = hf[0:min(WIN, npad), :]
                    src_hi = None if single_win else hf[hi_start:npad, :]
                klo_b, khi_b = int(k_lo[b]), int(k_hi[b])
                ktb = kt_b[b]
                seg8 = int(koff[b]) * 8
                msgs_lo = mpool.tile([P, klo_b, D], f32, tag="mlo",
                                     padded_shape=[P, int(k_lo.max()), D])
                nc.gpsimd.dma_gather(
                    out_ap=msgs_lo[:], in_ap=src_lo,
                    idxs_ap=idx_sb[:, seg8:seg8 + klo_b * 8],
                    num_idxs=klo_b * P, num_idxs_reg=klo_b * P, elem_size=D,
                    single_packet=False, queue_num=(2 * b) % 4)
                if khi_b:
                    msgs_hi = mpool.tile([P, khi_b, D], f32, tag="mhi",
                                         padded_shape=[P, int(k_hi.max()), D])
                    nc.gpsimd.dma_gather(
                        out_ap=msgs_hi[:], in_ap=src_hi,
                        idxs_ap=idx_sb[:, seg8 + klo_b * 8:seg8 + ktb * 8],
                        num_idxs=khi_b * P, num_idxs_reg=khi_b * P, elem_size=D,
                        single_packet=False, queue_num=(2 * b + 1) % 4)

                # ---- segment-sum: aggT[d, n] += sum_j msgs[j, d] * Emat[j, n]
                agg_ps = agg_pp.tile([P, P], f32, space="PSUM", tag="agg")
                for k in range(ktb):
                    col = int(koff[b]) + k
                    emat = epool.tile([P, P], f32, tag="emat")
                    nc.vector.tensor_tensor(
                        out=emat[:],
                        in0=dst_sb[:, col:col + 1].to_broadcast([P, P])[:],
                        in1=iota_sb[:],
                        op=mybir.AluOpType.is_equal)
                    rhs = (msgs_lo[:, k, :] if k < klo_b
                           else msgs_hi[:, k - klo_b, :])
                    nc.tensor.matmul(out=agg_ps[:], lhsT=rhs, rhs=emat[:],
                                     start=(k == 0), stop=(k == ktb - 1))

                # ---- zT = hT + aggT ; MLP (transposed); relu
                zT = zpool.tile([P, P], f32, tag="zT")
                nc.vector.tensor_add(zT[:], agg_ps[:],
                                     hT_cur[:, b * P:(b + 1) * P])
                mm1 = mlp_pp.tile([P, P], f32, space="PSUM", tag="mm1")
                nc.tensor.matmul(out=mm1[:], lhsT=w1_sb[l][:], rhs=zT[:],
                                 start=True, stop=True)
                a1T = zpool.tile([P, P], f32, tag="a1T")
                nc.scalar.activation(a1T[:], mm1[:], relu, bias=b1_sb[:, l:l + 1])
                mm2 = mlp_pp.tile([P, P], f32, space="PSUM", tag="mm2")
                nc.tensor.matmul(out=mm2[:], lhsT=w2_sb[l][:], rhs=a1T[:],
                                 start=True, stop=True)
                nc.scalar.activation(hT_nxt[:, b * P:(b + 1) * P], mm2[:], relu,
                                     bias=b2_sb[:, l:l + 1])

                # ---- natural-layout h block (for DRAM rows + pooling)
                hnew = zpool.tile([P, P], f32, tag="hnew")
                if SKIP_TR:
                    nc.vector.tensor_copy(hnew[:], hT_nxt[:, b * P:(b + 1) * P])
                else:
                    tr_ps = tr_pp.tile([P, P], f32, space="PSUM", tag="tr")
                    nc.tensor.transpose(out=tr_ps[:],
                                        in_=hT_nxt[:, b * P:(b + 1) * P],
                                        identity=ident_sb[:])
                    nc.vector.tensor_copy(hnew[:], tr_ps[:])
                nc.sync.dma_start(y_out[l][b * P:(b + 1) * P, :], hnew[:])
                if l < L - 1:
                    nc.sync.dma_start(hown[l][b * P:(b + 1) * P, :], hnew[:])

                # ---- pooling: pooled[g, d] += sum_n Bmat[n, g] * hnew[n, d]
                qsz = (bpc + 3) // 4
                if (l < L_RUN - 1 and not SKIP_COLL
                        and ((b + 1) % qsz == 0 or b == bpc - 1)):
                    r0 = (b // qsz) * qsz * P
                    r1 = min((b + 1) * P, npc)
                    nc.gpsimd.collective_compute(
                        "AllGather", mybir.AluOpType.bypass,
                        replica_groups=[list(range(NCORES))],
                        ins=[hown[l][r0:r1, :]],
                        outs=[hfull[l][NCORES * r0:NCORES * r1, :]])
                bmat = epool.tile([P, P], f32, tag="bmat")
                nc.vector.tensor_tensor(
                    out=bmat[:],
                    in0=bloc_sb[:, b:b + 1].to_broadcast([P, P])[:],
                    in1=iota_sb[:],
                    op=mybir.AluOpType.is_equal)
                nc.tensor.matmul(out=pooled_ps[:], lhsT=bmat[:], rhs=hnew[:],
                                 start=(b == 0) or POOL_NOACC,
                                 stop=(b == bpc - 1) or POOL_NOACC)

            pooled_sb = zpool.tile([P, D], f32, tag="pooled")
            nc.vector.tensor_copy(pooled_sb[:], pooled_ps[:])
            nc.sync.dma_start(pool_out[l][:], pooled_sb[:])

            if l < L_RUN - 1:
                if SKIP_COLL:
                    nc.sync.dma_start(hfull[l][0:npc, :], hown[l][:])
                else:
                    pass  # chunked allgather issued inside the block loop

    nc.finalize()
    return nc


# ---------------------------------------------------------------- entry
def kernel(x, edge_index, batch, W1, b1, W2, b2):
    global LAST_RESULT
    x = np.asarray(x, np.float32)
    edge_index = np.asarray(edge_index, np.int64)
    batch = np.asarray(batch, np.int64)
    W1 = np.ascontiguousarray(np.asarray(W1, np.float32))
    W2 = np.ascontiguousarray(np.asarray(W2, np.float32))
    b1 = np.asarray(b1, np.float32)
    b2 = np.asarray(b2, np.float32)

    n, d = x.shape
    assert d == D
    num_graphs = int(batch.max()) + 1
    npad = ((n + NCORES * P - 1) // (NCORES * P)) * (NCORES * P)
    npc = npad // NCORES

    src, dst = edge_index[0], edge_index[1]
    cores, k_lo, k_hi, bpc, single_win, hi_start, all_perm, qstarts = _prep(
        src, dst, batch, n, npad)

    nc = _build(npad, bpc, k_lo, k_hi, single_win, hi_start)

    x_pad = np.zeros((npad, D), np.float32)
    x_pad[:n] = x
    x_perm = np.zeros((npad, D), np.float32)
    x_perm[all_perm] = x_pad
    iota = np.tile(np.arange(P, dtype=np.float32), (P, 1))
    b1T = np.tile(b1.T[:, None, :], (1, 1, 1)).reshape(D, L)  # [D, L]
    b1T = np.ascontiguousarray(b1.T)                          # [D, L]
    b2T = np.ascontiguousarray(b2.T)

    in_maps = []
    for c in range(NCORES):
        cc = cores[c]
        xT = np.ascontiguousarray(x_pad[c * npc:(c + 1) * npc].T)
        in_maps.append(dict(x_full=x_perm, xT_own=xT, idx=cc['idx'],
                            dstl=cc['dstl'], bloc=cc['bloc'],
                            W1=W1, W2=W2, b1T=b1T, b2T=b2T, iota=iota))

    res = run_bass_kernel_spmd(nc, in_maps, list(range(NCORES)),
                               trace=bool(os.environ.get("BASS_TRACE")))
    LAST_RESULT = res

    xs = np.empty((L, n, D), np.float32)
    pooled = np.zeros((L, num_graphs, D), np.float32)
    for c in range(NCORES):
        r = res.results[c]
        lo, hi = c * npc, min((c + 1) * npc, n)
        for l in range(L):
            xs[l, lo:hi] = r[f"y{l}"][:hi - lo]
            g = cores[c]
            pooled[l, g['g_lo']:g['g_lo'] + g['n_graphs']] += \
                r[f"pool{l}"][:g['n_graphs']]
    xs_cat = np.concatenate(list(xs), axis=1)
    pooled_cat = np.concatenate(list(pooled), axis=1)
    return pooled_cat, xs_cat
